# revision 4
# baseline (speedup 1.0000x reference)
"""GCN message-passing kernel V2 for 8 trn2 NeuronCores.

Math:  out = segment_sum(h[edge_src], edge_dst) @ W_post + b_post,
       h = data @ W_pre + b_pre.
By linearity:
       out[d] = sum_{e: dst=d} (data[src_e] @ Wc) + deg[d]*(b_pre @ W_post)
                + b_post,   Wc = W_pre @ W_post  (folded on host).

V2 vs baseline: project data down to 64 features ON DEVICE first (PE, bf16),
store h rows f32 to a DRAM scratch in a partition-swizzled node order (so the
projection's [node-on-partition] output DMAs contiguously), then gather 256B
h rows per edge with dma_gather striped over 4 SWDGE queues (desc-gen runs on
a different Q7 core pair per queue -> ~3x faster than one queue).  Segment-sum
via one-hot matmuls (bf16) accumulating in PSUM per 128-dst block; bias terms
added with two rank-1 matmuls; output written transposed [64, 12544].

Sharding: dst-node shards of 12500 per core, fully independent, no
collectives. Self-contained: only numpy/ml_dtypes + concourse imports.
"""

import numpy as np
import ml_dtypes

import concourse.bacc as bacc
import concourse.mybir as mybir
import concourse.tile as tile
from concourse import library_config
from concourse.bass_utils import run_bass_kernel_spmd

F32 = mybir.dt.float32
BF16 = mybir.dt.bfloat16
I16 = mybir.dt.int16
NPBF = ml_dtypes.bfloat16


class Cfg:
    N = 100000          # nodes
    NPAD = 100352       # 49 * 2048
    NBATCH = 49         # projection batches of 2048 nodes
    DIN = 128
    DOUT = 64
    NC = 8              # cores
    SH = 12500          # dst nodes per core
    NB = 98             # dst blocks of 128 per core (12544 padded)
    G = 7               # blocks per group
    NG = 14             # groups (14*7 = 98)
    NW = 4              # src windows
    WS = 25088          # window size (int16-safe, NPAD/4)
    CU = 5              # 128-slot chunks per (block, window) cell

    def __init__(self, cu=5):
        self.CU = cu
        self.SPC = cu * 128             # slots per cell
        self.CALL = self.G * self.SPC   # slots per gather call (group-window)
        self.TOT = self.NB * self.NW * self.SPC  # slots per core


_DEFAULT_CFG = Cfg()


def preprocess(edge_src, edge_dst, cfg=_DEFAULT_CFG):
    """Per-core gather-index / dst-local / degree arrays (pure index math)."""
    src = np.asarray(edge_src).astype(np.int64)
    dst = np.asarray(edge_dst).astype(np.int64)

    # node -> h-table row (partition-swizzled projection layout):
    # batch c of 2048 nodes, node n = c*2048 + r lands at row
    # c*2048 + (r%128)*16 + r//128
    c = src // 2048
    r = src - c * 2048
    hrow = c * 2048 + (r % 128) * 16 + r // 128

    core = dst // cfg.SH
    dl = dst - core * cfg.SH
    blk = dl // 128
    loc = dl - blk * 128
    win = hrow // cfg.WS
    widx = hrow - win * cfg.WS

    grp = blk // cfg.G
    bi = blk - grp * cfg.G

    # slot = call_base(grp, win) + bi*640 + rank_within_cell
    cell = ((core * cfg.NB + blk) * cfg.NW + win)
    order = np.argsort(cell, kind="stable")
    cell_s = cell[order]
    counts = np.bincount(cell, minlength=cfg.NC * cfg.NB * cfg.NW)
    assert counts.max() <= cfg.SPC, (counts.max(), cfg.SPC)
    starts = np.zeros(cfg.NC * cfg.NB * cfg.NW, np.int64)
    starts[1:] = np.cumsum(counts)[:-1]
    rank_sorted = np.arange(len(src)) - starts[cell_s]
    rank = np.empty(len(src), np.int64)
    rank[order] = rank_sorted

    slot = ((grp * cfg.NW + win) * cfg.CALL + bi * cfg.SPC + rank)

    idx_all = np.zeros((cfg.NC, cfg.TOT), np.int16)
    loc_all = np.full((cfg.NC, cfg.TOT), -1.0, np.float32)
    idx_all[core, slot] = widx.astype(np.int16)
    loc_all[core, slot] = loc.astype(np.float32)

    ncalls = cfg.NG * cfg.NW  # 56
    cw = cfg.CALL // 16       # 280
    cl = cfg.CALL // 128      # 35

    # idx: per call wrap into 16 partitions, replicate 8x across 128
    w = idx_all.reshape(cfg.NC, ncalls, cw, 16).transpose(0, 1, 3, 2)
    w = np.tile(w, (1, 1, 8, 1))  # [NC, 56, 128, 280]
    idx_dram = np.ascontiguousarray(
        w.transpose(0, 2, 1, 3).reshape(cfg.NC, 128, ncalls * cw))

    ll = loc_all.reshape(cfg.NC, ncalls, cl, 128).transpose(0, 1, 3, 2)
    loc_dram = np.ascontiguousarray(
        ll.transpose(0, 2, 1, 3).reshape(cfg.NC, 128, ncalls * cl)).astype(NPBF)

    degs = np.bincount(dst, minlength=cfg.N).astype(np.float32)
    deg_dram = np.zeros((cfg.NC, 1, cfg.NB * 128), NPBF)
    for cc in range(cfg.NC):
        deg_dram[cc, 0, :cfg.SH] = degs[cc * cfg.SH:(cc + 1) * cfg.SH].astype(NPBF)

    return idx_dram, loc_dram, deg_dram


def build_program(cfg=_DEFAULT_CFG):
    nc = bacc.Bacc("TRN2", target_bir_lowering=False, debug=True,
                   num_swdge_queues=4)

    dataT = nc.dram_tensor("dataT", [128, cfg.NPAD], BF16, kind="ExternalInput")
    idxs = nc.dram_tensor("idxs", [128, cfg.NG * cfg.NW * cfg.CALL // 16], I16,
                          kind="ExternalInput")
    locs = nc.dram_tensor("locs", [128, cfg.NG * cfg.NW * cfg.CALL // 128], BF16,
                          kind="ExternalInput")
    deg = nc.dram_tensor("deg", [1, cfg.NB * 128], BF16, kind="ExternalInput")
    iota_in = nc.dram_tensor("iota", [128, 128], BF16, kind="ExternalInput")
    wc_in = nc.dram_tensor("wc", [cfg.DIN, cfg.DOUT], BF16, kind="ExternalInput")
    bpw_in = nc.dram_tensor("bpw", [1, cfg.DOUT], BF16, kind="ExternalInput")
    bpost_in = nc.dram_tensor("bpost", [1, cfg.DOUT], BF16, kind="ExternalInput")
    out = nc.dram_tensor("out", [cfg.DOUT, cfg.NB * 128], F32,
                         kind="ExternalOutput")

    with tile.TileContext(nc) as tc:
        nc.gpsimd.load_library(library_config.mlp)
        with (
            tc.tile_pool(name="consts", bufs=1) as cpool,
            tc.tile_pool(name="hdram", bufs=1, space="DRAM") as hpool,
        ):
            iota_sb = cpool.tile([128, 128], BF16)
            wc_sb = cpool.tile([cfg.DIN, cfg.DOUT], BF16)
            bpw_sb = cpool.tile([1, cfg.DOUT], BF16)
            bpost_sb = cpool.tile([1, cfg.DOUT], BF16)
            ones_sb = cpool.tile([1, 128], BF16)
            nc.sync.dma_start(out=iota_sb[:], in_=iota_in[:])
            nc.sync.dma_start(out=wc_sb[:], in_=wc_in[:])
            nc.sync.dma_start(out=bpw_sb[:], in_=bpw_in[:])
            nc.sync.dma_start(out=bpost_sb[:], in_=bpost_in[:])
            nc.vector.memset(ones_sb[:], 1.0)

            h_t = hpool.tile([cfg.NPAD, cfg.DOUT], F32)

            # ---- phase 1: h = dataT^T @ Wc, node-on-partition, swizzled ----
            with (
                tc.tile_pool(name="dtp", bufs=3) as dtp,
                tc.tile_pool(name="ps1", bufs=2, space="PSUM") as ps1,
                tc.tile_pool(name="hsb", bufs=3) as hsbp,
            ):
                for cb in range(cfg.NBATCH):
                    dt = dtp.tile([128, 2048], BF16)
                    nc.sync.dma_start(
                        out=dt[:], in_=dataT[:, cb * 2048:(cb + 1) * 2048])
                    hs = hsbp.tile([128, 16, cfg.DOUT], F32)
                    for half in range(2):
                        ps = ps1.tile([128, 512], F32, tag="proj")
                        for j in range(8):
                            ch = half * 8 + j
                            nc.tensor.matmul(
                                out=ps[:, j * 64:(j + 1) * 64],
                                lhsT=dt[:, ch * 128:(ch + 1) * 128],
                                rhs=wc_sb[:], start=True, stop=True)
                        nc.scalar.copy(hs[:, half * 8:(half + 1) * 8, :], ps[:])
                    nc.sync.dma_start(
                        out=h_t[cb * 2048:(cb + 1) * 2048, :], in_=hs[:])

            # ---- phase 2: gather h rows per edge, one-hot segment sum ----
            with (
                tc.tile_pool(name="idxp", bufs=6) as idxp,
                tc.tile_pool(name="locp", bufs=6) as locp,
                tc.tile_pool(name="msgs", bufs=4) as msgp,
                tc.tile_pool(name="msgb", bufs=4) as msgbp,
                tc.tile_pool(name="oh", bufs=4) as ohp,
                tc.tile_pool(name="psacc", bufs=cfg.G, space="PSUM") as psacc,
                tc.tile_pool(name="outsb", bufs=2) as outp,
                tc.tile_pool(name="degp", bufs=2) as degp,
            ):
                cw = cfg.CALL // 16    # 280
                cl = cfg.CALL // 128   # 35
                for grp in range(cfg.NG):
                    deg_t = degp.tile([1, cfg.G * 128], BF16)
                    nc.sync.dma_start(
                        out=deg_t[:],
                        in_=deg[:, grp * cfg.G * 128:(grp + 1) * cfg.G * 128])
                    accs = [psacc.tile([cfg.DOUT, 128], F32,
                                       name=f"acc{grp}_{i}", tag="acc")
                            for i in range(cfg.G)]
                    for w in range(cfg.NW):
                        call = grp * cfg.NW + w
                        idx_t = idxp.tile([128, cw], I16)
                        nc.sync.dma_start(
                            out=idx_t[:],
                            in_=idxs[:, call * cw:(call + 1) * cw])
                        loc_t = locp.tile([128, cl], BF16)
                        nc.sync.dma_start(
                            out=loc_t[:],
                            in_=locs[:, call * cl:(call + 1) * cl])
                        m_t = msgp.tile([128, cl, cfg.DOUT], F32)
                        nc.gpsimd.dma_gather(
                            m_t[:], h_t[w * cfg.WS:(w + 1) * cfg.WS, :],
                            idx_t[:], cfg.CALL, cfg.CALL, cfg.DOUT,
                            single_packet=False, queue_num=call % 4)
                        mb_t = msgbp.tile([128, cl, cfg.DOUT], BF16)
                        nc.scalar.copy(mb_t[:], m_t[:])
                        o_t = ohp.tile([128, cl, 128], BF16)
                        nc.vector.tensor_tensor(
                            out=o_t[:],
                            in0=loc_t[:].unsqueeze(2).broadcast_to([128, cl, 128]),
                            in1=iota_sb[:].unsqueeze(1).broadcast_to([128, cl, 128]),
                            op=mybir.AluOpType.is_equal)
                        for bi in range(cfg.G):
                            for cu in range(cfg.CU):
                                ch = bi * cfg.CU + cu
                                nc.tensor.matmul(
                                    out=accs[bi][:],
                                    lhsT=mb_t[:, ch, :],
                                    rhs=o_t[:, ch, :],
                                    start=(w == 0 and cu == 0), stop=False)
                    out_t = outp.tile([cfg.DOUT, cfg.G * 128], F32)
                    for bi in range(cfg.G):
                        nc.tensor.matmul(
                            out=accs[bi][:], lhsT=bpw_sb[:],
                            rhs=deg_t[:, bi * 128:(bi + 1) * 128],
                            start=False, stop=False)
                        nc.tensor.matmul(
                            out=accs[bi][:], lhsT=bpost_sb[:], rhs=ones_sb[:],
                            start=False, stop=True)
                        nc.scalar.copy(out_t[:, bi * 128:(bi + 1) * 128],
                                       accs[bi][:])
                    nc.sync.dma_start(
                        out=out[:, grp * cfg.G * 128:(grp + 1) * cfg.G * 128],
                        in_=out_t[:])
    nc.compile()
    return nc


_PROGRAM_CACHE = {}


def _get_program(cfg=_DEFAULT_CFG):
    key = (cfg.N, cfg.G, cfg.CU)
    if key not in _PROGRAM_CACHE:
        _PROGRAM_CACHE[key] = build_program(cfg)
    return _PROGRAM_CACHE[key]


def make_in_maps(data, edge_src, edge_dst, W_pre, b_pre, W_post, b_post,
                 cfg=_DEFAULT_CFG):
    idx_dram, loc_dram, deg_dram = preprocess(edge_src, edge_dst, cfg)
    data = np.asarray(data, dtype=np.float32)
    dataT = np.zeros((128, cfg.NPAD), NPBF)
    dataT[:, :cfg.N] = data.T.astype(NPBF)
    wc = (np.asarray(W_pre, np.float32) @ np.asarray(W_post, np.float32))
    bpw = (np.asarray(b_pre, np.float32) @ np.asarray(W_post, np.float32))
    iota = np.tile(np.arange(128, dtype=np.float32), (128, 1))
    in_maps = []
    for c in range(cfg.NC):
        in_maps.append({
            "dataT": dataT,
            "idxs": idx_dram[c],
            "locs": loc_dram[c],
            "deg": deg_dram[c],
            "iota": iota.astype(NPBF),
            "wc": wc.astype(NPBF),
            "bpw": bpw.reshape(1, cfg.DOUT).astype(NPBF),
            "bpost": np.asarray(b_post, np.float32).reshape(1, cfg.DOUT).astype(NPBF),
        })
    return in_maps


def _needed_cu(edge_src, edge_dst, base=Cfg):
    src = np.asarray(edge_src).astype(np.int64)
    dst = np.asarray(edge_dst).astype(np.int64)
    c = src // 2048
    r = src - c * 2048
    hrow = c * 2048 + (r % 128) * 16 + r // 128
    core = dst // base.SH
    dl = dst - core * base.SH
    blk = dl // 128
    win = hrow // base.WS
    cell = ((core * base.NB + blk) * base.NW + win)
    counts = np.bincount(cell, minlength=base.NC * base.NB * base.NW)
    return max(5, -(-int(counts.max()) // 128))


def kernel(data, edge_src, edge_dst, W_pre, b_pre, W_post, b_post):
    cfg = Cfg(_needed_cu(edge_src, edge_dst))
    nc = _get_program(cfg)
    in_maps = make_in_maps(data, edge_src, edge_dst, W_pre, b_pre, W_post,
                           b_post, cfg)
    res = run_bass_kernel_spmd(nc, in_maps, list(range(cfg.NC)), trace=False)
    out = np.empty((cfg.N, cfg.DOUT), np.float32)
    for c in range(cfg.NC):
        out[c * cfg.SH:(c + 1) * cfg.SH, :] = res.results[c]["out"][:, :cfg.SH].T
    return out
